# revision 3
# baseline (speedup 1.0000x reference)
"""Trainium2 8-core SPMD kernel for MQA attention with relative position bias.

v6 (= v5 + front/pipeline tuning): independent cores, no collective.
  - DMA order puts x chunk 0/1 at the head of the sync ring and Wkv at
    the head of the scalar ring (the first kv matmul gates everything);
    bias chunks alternate rings with 4 buffers.
  - the 4 per-chunk v-transposes collect in ONE single-bank PSUM tile
    with ONE DVE copy to V65 (the per-tile copies churned the 2-slot
    PSUM pool and kept the PE gappy/cold through the projection phase).
  - queries sharded: core c owns q-tiles {c, 15-c} per batch (512 tok/core),
    attention padded to a uniform 8+16 (slot,j-tile) pairs per batch.
  - k/v REPLICATED: each core projects k/v for all 4096 tokens (bf16 x).
  - every DMA moves a [128, contiguous] block (host pre-chunks x, weights
    and bias into partition-major layouts): one run per partition keeps
    the HWDGE issue slices short (multi-run APs measured 6-16us each and
    blocked the engine stream).
  - x and weights + output stream on the Sync HWDGE ring; the 12.6MB
    bias streams as 1MB 4-pair chunks on the Scalar ring.
  - MULTIPLICATIVE bias: host ships expB = exp(bias) (0 for masked
    entries); the DVE multiplies it into the exp'd scores, replacing the
    PE identity-injection matmuls (and their LDWEIGHTS churn) entirely.
  - scores run as TWO CONCURRENT K=64 row-tiles (heads 0-3 in rows 0:63,
    heads 4-7 in rows 64:127, separate PSUM banks).
  - softmax 1/l = exp(-ln(l)) on ACT; the activation-table map is patched
    so Exp resolves to the natural_log_exp_and_others set (otherwise the
    Ln<->Exp alternation reloads ACT tables twice per slot, ~2.7us each).
"""

import os
import sys

import numpy as np

sys.path.insert(0, "/opt/trn_rl_repo")

import ml_dtypes

BF16 = ml_dtypes.bfloat16

# ---- problem constants (hardcoded per the harness contract) ----
B = 2
N = 2048
DIM = 1024
HEADS = 8
DH = 64
INNER = HEADS * DH  # 512
P = 128
NT = N // P  # 16 q/k tiles per batch
EXTA, EXTB = 8, 16  # j-tile extents for slot A (q-tile c) / slot B (15-c)
NPAIR = EXTA + EXTB  # 24 (slot, j-tile) pairs per batch per core
NCHUNK = NPAIR // 4  # 6 bias chunks per batch (4 pairs each)
NCORES = 8
TOK_OWN = 4 * P  # 512 own tokens per core
BN = B * N  # 4096 total tokens
NTC = BN // 512  # 8 token-chunks for the kv projection
NEG = -1.0e30  # masked logit (pre-scale), exp -> 0

_CACHE = {}


def _q_tiles(c):
    return [c, NT - 1 - c]


def _patch_act_tables():
    """Make Exp resolve to the set that also holds Ln (set 6,
    natural_log_exp_and_others) so the per-slot Ln -> Exp normalize does
    not force ACT table reloads. Set ids stay positional (walrus maps the
    id into act_info.json), only the selection input is filtered."""
    import functools

    import concourse.bacc as bacc_mod
    import concourse.hw_specs as hw_specs_mod
    import concourse.mybir as mybir

    if getattr(hw_specs_mod, "_exp_set_patched", False):
        return
    orig = hw_specs_mod.get_activation_tables

    @functools.cache
    def patched(arch):
        tabs = orig(arch)
        exp = mybir.ActivationFunctionType.Exp
        out = {}
        for name, fns in tabs.items():
            if exp in fns and "natural_log" not in name:
                fns = fns - {exp}
            out[name] = fns
        return out

    hw_specs_mod.get_activation_tables = patched
    hw_specs_mod._exp_set_patched = True
    if hasattr(bacc_mod, "get_activation_tables"):
        bacc_mod.get_activation_tables = patched


def build_graph():
    import concourse.bass as bass
    import concourse.bacc as bacc
    import concourse.mybir as mybir
    import concourse.tile as tile

    _patch_act_tables()

    dt = mybir.dt
    f32, f32r, bf16 = dt.float32, dt.float32r, dt.bfloat16
    AF = mybir.ActivationFunctionType

    nc = bacc.Bacc(None, target_bir_lowering=False)

    # ---- I/O (all pre-chunked on host to [128, contiguous] blocks) ----
    xTa_t = nc.dram_tensor("xTaC", [NTC, P, 8 * 512], bf16, kind="ExternalInput")
    xTo_t = nc.dram_tensor("xToC", [P, 8 * TOK_OWN], bf16, kind="ExternalInput")
    Wq_t = nc.dram_tensor("WqC", [P, 8 * INNER], bf16, kind="ExternalInput")
    Wkv_t = nc.dram_tensor("WkvC", [P, 8 * 2 * DH], bf16, kind="ExternalInput")
    Wo_t = nc.dram_tensor("WoC", [P, 4 * DIM], bf16, kind="ExternalInput")
    bo_t = nc.dram_tensor("bo", [1, DIM], f32r, kind="ExternalInput")
    ident_t = nc.dram_tensor("ident", [P, P], bf16, kind="ExternalInput")
    ones_r_t = nc.dram_tensor("ones_r", [1, P], f32r, kind="ExternalInput")
    ones_bn_t = nc.dram_tensor("ones_bn", [1, BN], bf16, kind="ExternalInput")
    # biasC[b, chunk, j, (4 pairs x h x q)]: exp(bias), 0 where masked
    bias_t = nc.dram_tensor(
        "biasC", [B, NCHUNK, P, 4 * HEADS * P], bf16, kind="ExternalInput"
    )
    out_t = nc.dram_tensor("out", [TOK_OWN, DIM], f32, kind="ExternalOutput")

    with tile.TileContext(nc) as tc:
        with (
            tc.tile_pool(name="const", bufs=1) as cpool,
            tc.tile_pool(name="bias", bufs=4) as bpool,
            tc.tile_pool(name="pt", bufs=3) as ptpool,
            tc.tile_pool(name="at", bufs=2) as atpool,
            tc.tile_pool(name="ob", bufs=2) as obpool,
            tc.tile_pool(name="ps", bufs=2, space="PSUM") as pspool,
        ):
            # ---- constants / weights into SBUF ----
            # sync ring leads with x chunk 0/1 (first kv matmul gates
            # everything); scalar ring leads with Wkv
            xTa_sb = cpool.tile([P, NTC * 8 * 512], bf16, tag="xTa_sb")

            def xta_dma(tc_i):
                nc.sync.dma_start(
                    out=xTa_sb[:, tc_i * 4096 : (tc_i + 1) * 4096],
                    in_=xTa_t[tc_i],
                )

            xta_dma(0)
            Wkv_sb = cpool.tile([P, 8 * 2 * DH], bf16, tag="Wkv_sb")
            nc.scalar.dma_start(out=Wkv_sb[:], in_=Wkv_t[:])
            xta_dma(1)
            ident_sb = cpool.tile([P, P], bf16, tag="ident_sb")
            nc.sync.dma_start(out=ident_sb[:], in_=ident_t[:])
            ones128 = cpool.tile([1, P], f32r, tag="ones128")
            nc.sync.dma_start(out=ones128[:], in_=ones_r_t[:])
            bo_sb = cpool.tile([1, DIM], f32r, tag="bo_sb")
            nc.sync.dma_start(out=bo_sb[:], in_=bo_t[:])
            kT2 = cpool.tile([P, BN], bf16, tag="kT2")
            vT65 = cpool.tile([DH + 1, BN], bf16, tag="vT65")
            nc.scalar.dma_start(out=vT65[DH : DH + 1, :], in_=ones_bn_t[:])
            V65 = cpool.tile([P, B * NT * (DH + 1)], bf16, tag="V65")
            qT_sb = cpool.tile([P, HEADS * TOK_OWN], bf16, tag="qT_sb")
            xTo_sb = cpool.tile([P, 8 * TOK_OWN], bf16, tag="xTo_sb")
            nc.scalar.dma_start(out=xTo_sb[:], in_=xTo_t[:])
            Wq_sb = cpool.tile([P, 8 * INNER], bf16, tag="Wq_sb")
            nc.sync.dma_start(out=Wq_sb[:], in_=Wq_t[:])
            xta_dma(2)
            xta_dma(3)
            Wo_sb = cpool.tile([P, 4 * DIM], bf16, tag="Wo_sb")
            nc.sync.dma_start(out=Wo_sb[:], in_=Wo_t[:])
            for tc_i in range(4, NTC):
                xta_dma(tc_i)

            # ---- k/v projection chunks (512 tokens each) ----
            def kv_chunk(tc_i):
                kvps = pspool.tile([P, 512], f32, tag="sT", name=f"kv{tc_i}")
                for fc in range(8):
                    nc.tensor.matmul(
                        kvps[:, :],
                        Wkv_sb[:, fc * 2 * DH : (fc + 1) * 2 * DH],
                        xTa_sb[:, tc_i * 4096 + fc * 512 : tc_i * 4096 + (fc + 1) * 512],
                        start=(fc == 0),
                        stop=(fc == 7),
                    )
                nc.vector.tensor_copy(
                    vT65[0:DH, tc_i * 512 : (tc_i + 1) * 512], kvps[DH:P, :]
                )
                nc.vector.tensor_copy(
                    kT2[0:DH, tc_i * 512 : (tc_i + 1) * 512], kvps[0:DH, :]
                )
                nc.vector.tensor_copy(
                    kT2[DH:P, tc_i * 512 : (tc_i + 1) * 512],
                    kT2[0:DH, tc_i * 512 : (tc_i + 1) * 512],
                )
                # all 4 v-transposes into ONE single-bank PSUM tile, one copy
                vtp = pspool.tile(
                    [P, 4 * (DH + 1)], f32, tag="sT", name=f"vt{tc_i}"
                )
                for i in range(4):
                    g = tc_i * 4 + i  # global j-tile index (b*NT + jt)
                    nc.tensor.matmul(
                        vtp[:, i * (DH + 1) : (i + 1) * (DH + 1)],
                        vT65[:, g * P : (g + 1) * P],
                        ident_sb[0 : DH + 1, 0 : DH + 1],
                        start=True,
                        stop=True,
                        skip_group_check=True,
                    )
                nc.vector.tensor_copy(
                    V65[:, tc_i * 4 * (DH + 1) : (tc_i + 1) * 4 * (DH + 1)],
                    vtp[:, :],
                )

            def q_chunk(hp):
                qps = pspool.tile([P, TOK_OWN], f32, tag="sT", name=f"q{hp}")
                for fc in range(8):
                    nc.tensor.matmul(
                        qps[:, :],
                        Wq_sb[:, fc * INNER + hp * P : fc * INNER + (hp + 1) * P],
                        xTo_sb[:, fc * TOK_OWN : (fc + 1) * TOK_OWN],
                        start=(fc == 0),
                        stop=(fc == 7),
                    )
                nc.vector.tensor_copy(
                    qT_sb[0:DH, (2 * hp) * TOK_OWN : (2 * hp + 1) * TOK_OWN],
                    qps[0:DH, :],
                )
                nc.vector.tensor_copy(
                    qT_sb[0:DH, (2 * hp + 1) * TOK_OWN : (2 * hp + 2) * TOK_OWN],
                    qps[DH:P, :],
                )

            kv_chunk(0)
            kv_chunk(1)
            q_chunk(0)
            q_chunk(1)
            kv_chunk(2)
            q_chunk(2)
            q_chunk(3)
            kv_chunk(3)

            # heads 4-7 duplicated into partitions 64:127 (for the
            # concurrent row-tile scores matmul)
            nc.vector.tensor_copy(
                qT_sb[DH:P, 0 : 4 * TOK_OWN], qT_sb[0:DH, 4 * TOK_OWN : 8 * TOK_OWN]
            )

            # batch-1 kv chunks injected between early batch-0 pairs
            late_kv = {8: 4, 12: 5, 16: 6, 20: 7}

            # ---- attention + output projection per (batch, slot) ----
            qT3 = qT_sb[:, :].rearrange("p (h t) -> p h t", h=HEADS)
            pair_ctr = 0
            for b in range(B):
                for sl, ext in ((0, EXTA), (1, EXTB)):
                    qcol = (2 * b + sl) * P  # q columns in qT/attnT order
                    pv = pspool.tile(
                        [P, HEADS * P], f32, tag="pv", name=f"pv{b}{sl}"
                    )
                    for jt in range(ext):
                        if pair_ctr in late_kv:
                            kv_chunk(late_kv[pair_ctr])
                        pair_ctr += 1
                        pair = jt if sl == 0 else EXTA + jt
                        # bias arrives in 1MB chunks of 4 consecutive pairs
                        # (scalar ring; [128, 4096] contiguous per chunk)
                        if pair % 4 == 0:
                            ci = b * NCHUNK + pair // 4
                            bias_sb = bpool.tile(
                                [P, 4 * HEADS * P], bf16, tag="bias"
                            )
                            eng = nc.scalar if ci % 2 == 0 else nc.sync
                            eng.dma_start(
                                out=bias_sb[:], in_=bias_t[b, pair // 4]
                            )
                        boff = (pair % 4) * HEADS * P
                        sT = pspool.tile(
                            [P, HEADS * P], f32, tag="sT", name=f"sT{b}{sl}{jt}"
                        )
                        kcol = (b * NT + jt) * P
                        # scores: two CONCURRENT K=64 row-tiles (different
                        # PSUM banks; lhsT base_partition picks the row tile)
                        nc.tensor.matmul(
                            sT[:, 0:512],
                            kT2[0:DH, kcol : kcol + P],
                            qT3[0:DH, 0:4, qcol : qcol + P],
                            start=True,
                            stop=True,
                            skip_group_check=True,
                        )
                        nc.tensor.matmul(
                            sT[:, 512:1024],
                            kT2[DH:P, kcol : kcol + P],
                            qT3[DH:P, 0:4, qcol : qcol + P],
                            start=True,
                            stop=True,
                            skip_group_check=True,
                        )
                        # softmax numerator: P^T = exp(sT/8) * expB in bf16
                        pt_sb = ptpool.tile([P, HEADS * P], bf16, tag="pt")
                        nc.scalar.activation(
                            pt_sb[:, :], sT[:, :], AF.Exp, scale=0.125
                        )
                        nc.vector.tensor_mul(
                            pt_sb[:, :],
                            pt_sb[:, :],
                            bias_sb[:, boff : boff + HEADS * P],
                        )
                        # PV: pv[0:65, :] += V65.T @ P^T (denominator in row 64)
                        g = (b * NT + jt) * (DH + 1)
                        for half in range(2):
                            nc.tensor.matmul(
                                pv[0 : DH + 1, half * 512 : (half + 1) * 512],
                                V65[:, g : g + DH + 1],
                                pt_sb[:, half * 512 : (half + 1) * 512],
                                start=(jt == 0),
                                stop=(jt == ext - 1),
                                skip_group_check=True,
                            )

                    # ---- normalize: attnT = pv[0:64] * (1/l) ----
                    # 1/l via ACT: exp(-log(l)) — table patch keeps both fns
                    # in the loaded set
                    recip = cpool.tile(
                        [1, HEADS * P], f32, name=f"rc{b}{sl}", tag="recip", bufs=2
                    )
                    lg = cpool.tile(
                        [1, HEADS * P], f32, name=f"lg{b}{sl}", tag="lg", bufs=2
                    )
                    nc.scalar.activation(lg[:, :], pv[DH : DH + 1, :], AF.Ln)
                    nc.scalar.activation(recip[:, :], lg[:, :], AF.Exp, scale=-1.0)
                    bc_sb = ptpool.tile(
                        [DH, HEADS * P], f32, tag="bc", name=f"bc{b}{sl}", bufs=2
                    )
                    nc.gpsimd.partition_broadcast(bc_sb[:, :], recip[:, :])
                    attnT = atpool.tile([P, HEADS * P], bf16, tag="at")
                    for half in range(2):
                        fs = slice(half * 512, (half + 1) * 512)
                        nc.vector.tensor_mul(attnT[0:DH, fs], pv[0:DH, fs], bc_sb[:, fs])
                    # shifted duplicate: rows 64:128 col g*128 hold head g+1
                    nc.vector.tensor_copy(
                        attnT[DH:P, 0 : 7 * P], attnT[0:DH, P : HEADS * P]
                    )

                    # ---- output projection for this slot's 128 tokens ----
                    orow = (2 * b + sl) * P
                    for half in range(2):
                        fs = slice(half * 512, (half + 1) * 512)
                        ops = pspool.tile(
                            [P, 512], f32, tag="pv", name=f"op{b}{sl}{half}"
                        )
                        nc.tensor.matmul(
                            ops[:, :], ones128[:, :], bo_sb[:, fs], start=True, stop=False
                        )
                        for hp in range(4):
                            nc.tensor.matmul(
                                ops[:, :],
                                attnT[:, 2 * hp * P : (2 * hp + 1) * P],
                                Wo_sb[:, hp * DIM + half * 512 : hp * DIM + (half + 1) * 512],
                                start=False,
                                stop=(hp == 3),
                            )
                        ob_sb = obpool.tile([P, 512], f32, tag="ob")
                        nc.vector.tensor_copy(ob_sb[:, :], ops[:, :])
                        nc.sync.dma_start(
                            out=out_t[orow : orow + P, half * 512 : (half + 1) * 512],
                            in_=ob_sb[:, :],
                        )

    nc.compile()
    return nc


def prep_inputs(x, rel_pos_bias, Wq, Wkv, Wo, bo):
    """Build the 8 per-core input maps (host-side sharding/marshalling)."""
    x = np.asarray(x, dtype=np.float32)
    rel_pos_bias = np.asarray(rel_pos_bias, dtype=np.float32)
    bo = np.asarray(bo, dtype=np.float32).reshape(1, DIM)

    def chunk_pm(w, nf):
        # [nf*128, C] -> [128, nf*C] (partition-major chunks)
        w = np.asarray(w, dtype=np.float32)
        c = w.shape[1]
        return np.ascontiguousarray(
            w.reshape(nf, P, c).transpose(1, 0, 2).reshape(P, nf * c)
        ).astype(BF16)

    WqC = chunk_pm(Wq, 8)
    WkvC = chunk_pm(Wkv, 8)
    WoC = chunk_pm(Wo, 4)
    xT = np.concatenate([x[b].T for b in range(B)], axis=1)  # [1024, 4096]
    # [tc, p, fc*512+t] = xT[fc*128+p, tc*512+t]
    xTaC = np.ascontiguousarray(
        xT.reshape(8, P, 8, 512).transpose(2, 1, 0, 3).reshape(NTC, P, 8 * 512)
    ).astype(BF16)
    ident = np.eye(P, dtype=BF16)
    ones_r = np.ones((1, P), np.float32)
    ones_bn = np.ones((1, BN), dtype=BF16)

    ji = np.arange(N)  # global key index
    in_maps = []
    for c in range(NCORES):
        tiles = _q_tiles(c)
        # own tokens, order [b0A, b0B, b1A, b1B]
        xs = [x[b, t * P : (t + 1) * P, :] for b in range(B) for t in tiles]
        xToC = chunk_pm(np.concatenate(xs, axis=0).T, 8)

        biasT = np.zeros((B, NPAIR, P, HEADS, P), dtype=np.float32)
        for b in range(B):
            for sl, (t, ext) in enumerate(zip(tiles, (EXTA, EXTB))):
                qg = t * P + np.arange(P)  # global q index [128]
                nj = ext * P
                # [h, q, j] -> [jt, j, h, q], expB = exp(bias), 0 if masked
                blk = rel_pos_bias[:, t * P : (t + 1) * P, :nj]
                blk = np.exp(blk.reshape(HEADS, P, ext, P).transpose(2, 3, 0, 1))
                m = ji[:nj, None] > qg[None, :]  # [j, q] masked
                blk = np.where(
                    m.reshape(ext, P, 1, P).repeat(HEADS, axis=2)[:, :, :HEADS, :],
                    0.0,
                    blk,
                )
                base = 0 if sl == 0 else EXTA
                biasT[b, base : base + ext] = blk
        # [b, pair, j, h, q] -> [b, chunk, j, (pair%4, h, q)]
        biasC = np.ascontiguousarray(
            biasT.reshape(B, NCHUNK, 4, P, HEADS, P)
            .transpose(0, 1, 3, 2, 4, 5)
            .reshape(B, NCHUNK, P, 4 * HEADS * P)
        ).astype(BF16)
        in_maps.append(
            {
                "xTaC": xTaC,
                "xToC": xToC,
                "WqC": WqC,
                "WkvC": WkvC,
                "WoC": WoC,
                "bo": bo,
                "ident": ident,
                "ones_r": ones_r,
                "ones_bn": ones_bn,
                "biasC": biasC,
            }
        )
    return in_maps


def assemble(outs):
    """outs: list of 8 [512, 1024] arrays -> full [2, 2048, 1024]."""
    full = np.empty((B, N, DIM), dtype=np.float32)
    for c in range(NCORES):
        o = np.asarray(outs[c])
        for b in range(B):
            for sl, t in enumerate(_q_tiles(c)):
                full[b, t * P : (t + 1) * P, :] = o[
                    (2 * b + sl) * P : (2 * b + sl + 1) * P
                ]
    return full


def kernel(**inputs):
    from concourse.bass_utils import run_bass_kernel_spmd

    if "nc" not in _CACHE:
        _CACHE["nc"] = build_graph()
    nc = _CACHE["nc"]
    in_maps = prep_inputs(
        inputs["x"], inputs["rel_pos_bias"], inputs["Wq"], inputs["Wkv"],
        inputs["Wo"], inputs["bo"],
    )
    res = run_bass_kernel_spmd(
        nc, in_maps, core_ids=list(range(NCORES)),
        trace=bool(int(os.environ.get("KERNEL_TRACE", "0"))),
    )
    _CACHE["last_results"] = res
    return assemble([r["out"] for r in res.results])


# revision 4
# speedup vs baseline: 1.1623x; 1.1623x over previous
"""Trainium2 8-core SPMD kernel for MQA attention with relative position bias.

v7 (= v6 + software-pipelined pair loop): independent cores, no collective.
  - the PE stream is software-pipelined one pair deep: scores for pair
    i+1 issue before the PV matmuls of pair i, so the PE never waits for
    the exp(ACT) -> *expB(DVE) chain of the current pair; each slot's
    normalize + output projection is deferred two pairs into the next
    slot for the same reason.
  - DMA order puts x chunk 0/1 at the head of the sync ring and Wkv at
    the head of the scalar ring (the first kv matmul gates everything);
    bias chunks alternate rings with 4 buffers.
  - the 4 per-chunk v-transposes collect in ONE single-bank PSUM tile
    with ONE DVE copy to V65 (the per-tile copies churned the 2-slot
    PSUM pool and kept the PE gappy/cold through the projection phase).
  - queries sharded: core c owns q-tiles {c, 15-c} per batch (512 tok/core),
    attention padded to a uniform 8+16 (slot,j-tile) pairs per batch.
  - k/v REPLICATED: each core projects k/v for all 4096 tokens (bf16 x).
  - every DMA moves a [128, contiguous] block (host pre-chunks x, weights
    and bias into partition-major layouts): one run per partition keeps
    the HWDGE issue slices short (multi-run APs measured 6-16us each and
    blocked the engine stream).
  - x and weights + output stream on the Sync HWDGE ring; the 12.6MB
    bias streams as 1MB 4-pair chunks on the Scalar ring.
  - MULTIPLICATIVE bias: host ships expB = exp(bias) (0 for masked
    entries); the DVE multiplies it into the exp'd scores, replacing the
    PE identity-injection matmuls (and their LDWEIGHTS churn) entirely.
  - scores run as TWO CONCURRENT K=64 row-tiles (heads 0-3 in rows 0:63,
    heads 4-7 in rows 64:127, separate PSUM banks).
  - softmax 1/l = exp(-ln(l)) on ACT; the activation-table map is patched
    so Exp resolves to the natural_log_exp_and_others set (otherwise the
    Ln<->Exp alternation reloads ACT tables twice per slot, ~2.7us each).
"""

import os
import sys

import numpy as np

sys.path.insert(0, "/opt/trn_rl_repo")

import ml_dtypes

BF16 = ml_dtypes.bfloat16

# ---- problem constants (hardcoded per the harness contract) ----
B = 2
N = 2048
DIM = 1024
HEADS = 8
DH = 64
INNER = HEADS * DH  # 512
P = 128
NT = N // P  # 16 q/k tiles per batch
EXTA, EXTB = 8, 16  # j-tile extents for slot A (q-tile c) / slot B (15-c)
NPAIR = EXTA + EXTB  # 24 (slot, j-tile) pairs per batch per core
NCHUNK = NPAIR // 4  # 6 bias chunks per batch (4 pairs each)
NCORES = 8
TOK_OWN = 4 * P  # 512 own tokens per core
BN = B * N  # 4096 total tokens
NTC = BN // 512  # 8 token-chunks for the kv projection
NEG = -1.0e30  # masked logit (pre-scale), exp -> 0

_CACHE = {}


def _q_tiles(c):
    return [c, NT - 1 - c]


def _patch_act_tables():
    """Make Exp resolve to the set that also holds Ln (set 6,
    natural_log_exp_and_others) so the per-slot Ln -> Exp normalize does
    not force ACT table reloads. Set ids stay positional (walrus maps the
    id into act_info.json), only the selection input is filtered."""
    import functools

    import concourse.bacc as bacc_mod
    import concourse.hw_specs as hw_specs_mod
    import concourse.mybir as mybir

    if getattr(hw_specs_mod, "_exp_set_patched", False):
        return
    orig = hw_specs_mod.get_activation_tables

    @functools.cache
    def patched(arch):
        tabs = orig(arch)
        exp = mybir.ActivationFunctionType.Exp
        out = {}
        for name, fns in tabs.items():
            if exp in fns and "natural_log" not in name:
                fns = fns - {exp}
            out[name] = fns
        return out

    hw_specs_mod.get_activation_tables = patched
    hw_specs_mod._exp_set_patched = True
    if hasattr(bacc_mod, "get_activation_tables"):
        bacc_mod.get_activation_tables = patched


def build_graph():
    import concourse.bass as bass
    import concourse.bacc as bacc
    import concourse.mybir as mybir
    import concourse.tile as tile

    try:
        _patch_act_tables()
    except Exception:
        pass  # degrade: extra ACT table reloads, still correct

    dt = mybir.dt
    f32, f32r, bf16 = dt.float32, dt.float32r, dt.bfloat16
    AF = mybir.ActivationFunctionType

    nc = bacc.Bacc(None, target_bir_lowering=False)

    # ---- I/O (all pre-chunked on host to [128, contiguous] blocks) ----
    xTa_t = nc.dram_tensor("xTaC", [NTC, P, 8 * 512], bf16, kind="ExternalInput")
    xTo_t = nc.dram_tensor("xToC", [P, 8 * TOK_OWN], bf16, kind="ExternalInput")
    Wq_t = nc.dram_tensor("WqC", [P, 8 * INNER], bf16, kind="ExternalInput")
    Wkv_t = nc.dram_tensor("WkvC", [P, 8 * 2 * DH], bf16, kind="ExternalInput")
    Wo_t = nc.dram_tensor("WoC", [P, 4 * DIM], bf16, kind="ExternalInput")
    bo_t = nc.dram_tensor("bo", [1, DIM], f32r, kind="ExternalInput")
    ident_t = nc.dram_tensor("ident", [P, P], bf16, kind="ExternalInput")
    ones_r_t = nc.dram_tensor("ones_r", [1, P], f32r, kind="ExternalInput")
    ones_bn_t = nc.dram_tensor("ones_bn", [1, BN], bf16, kind="ExternalInput")
    # biasC[b, chunk, j, (4 pairs x h x q)]: exp(bias), 0 where masked
    bias_t = nc.dram_tensor(
        "biasC", [B, NCHUNK, P, 4 * HEADS * P], bf16, kind="ExternalInput"
    )
    out_t = nc.dram_tensor("out", [TOK_OWN, DIM], f32, kind="ExternalOutput")

    with tile.TileContext(nc) as tc:
        with (
            tc.tile_pool(name="const", bufs=1) as cpool,
            tc.tile_pool(name="bias", bufs=4) as bpool,
            tc.tile_pool(name="pt", bufs=4) as ptpool,
            tc.tile_pool(name="at", bufs=2) as atpool,
            tc.tile_pool(name="ob", bufs=2) as obpool,
            tc.tile_pool(name="ps", bufs=2, space="PSUM") as pspool,
        ):
            # ---- constants / weights into SBUF ----
            # sync ring leads with x chunk 0/1 (first kv matmul gates
            # everything); scalar ring leads with Wkv
            xTa_sb = cpool.tile([P, NTC * 8 * 512], bf16, tag="xTa_sb")

            def xta_dma(tc_i, split=False):
                if split:
                    for h in range(2):
                        nc.sync.dma_start(
                            out=xTa_sb[
                                :,
                                tc_i * 4096 + h * 2048 : tc_i * 4096 + (h + 1) * 2048,
                            ],
                            in_=xTa_t[tc_i, :, h * 2048 : (h + 1) * 2048],
                        )
                else:
                    nc.sync.dma_start(
                        out=xTa_sb[:, tc_i * 4096 : (tc_i + 1) * 4096],
                        in_=xTa_t[tc_i],
                    )

            xta_dma(0, split=True)
            Wkv_sb = cpool.tile([P, 8 * 2 * DH], bf16, tag="Wkv_sb")
            nc.scalar.dma_start(out=Wkv_sb[:], in_=Wkv_t[:])
            xta_dma(1, split=True)
            ident_sb = cpool.tile([P, P], bf16, tag="ident_sb")
            nc.sync.dma_start(out=ident_sb[:], in_=ident_t[:])
            ones128 = cpool.tile([1, P], f32r, tag="ones128")
            nc.sync.dma_start(out=ones128[:], in_=ones_r_t[:])
            bo_sb = cpool.tile([1, DIM], f32r, tag="bo_sb")
            nc.sync.dma_start(out=bo_sb[:], in_=bo_t[:])
            kT2 = cpool.tile([P, BN], bf16, tag="kT2")
            vT65 = cpool.tile([DH + 1, BN], bf16, tag="vT65")
            nc.scalar.dma_start(out=vT65[DH : DH + 1, :], in_=ones_bn_t[:])
            V65 = cpool.tile([P, B * NT * (DH + 1)], bf16, tag="V65")
            qT_sb = cpool.tile([P, HEADS * TOK_OWN], bf16, tag="qT_sb")
            xTo_sb = cpool.tile([P, 8 * TOK_OWN], bf16, tag="xTo_sb")
            nc.scalar.dma_start(out=xTo_sb[:], in_=xTo_t[:])
            Wq_sb = cpool.tile([P, 8 * INNER], bf16, tag="Wq_sb")
            nc.sync.dma_start(out=Wq_sb[:], in_=Wq_t[:])
            xta_dma(2)
            xta_dma(3)
            Wo_sb = cpool.tile([P, 4 * DIM], bf16, tag="Wo_sb")
            nc.sync.dma_start(out=Wo_sb[:], in_=Wo_t[:])
            for tc_i in range(4, NTC):
                xta_dma(tc_i)

            # ---- k/v projection chunks (512 tokens each) ----
            def kv_chunk(tc_i):
                kvps = pspool.tile([P, 512], f32, tag="sT", name=f"kv{tc_i}")
                for fc in range(8):
                    nc.tensor.matmul(
                        kvps[:, :],
                        Wkv_sb[:, fc * 2 * DH : (fc + 1) * 2 * DH],
                        xTa_sb[:, tc_i * 4096 + fc * 512 : tc_i * 4096 + (fc + 1) * 512],
                        start=(fc == 0),
                        stop=(fc == 7),
                    )
                nc.vector.tensor_copy(
                    vT65[0:DH, tc_i * 512 : (tc_i + 1) * 512], kvps[DH:P, :]
                )
                nc.vector.tensor_copy(
                    kT2[0:DH, tc_i * 512 : (tc_i + 1) * 512], kvps[0:DH, :]
                )
                nc.vector.tensor_copy(
                    kT2[DH:P, tc_i * 512 : (tc_i + 1) * 512],
                    kT2[0:DH, tc_i * 512 : (tc_i + 1) * 512],
                )
                # all 4 v-transposes into ONE single-bank PSUM tile, one copy
                vtp = pspool.tile(
                    [P, 4 * (DH + 1)], f32, tag="sT", name=f"vt{tc_i}"
                )
                for i in range(4):
                    g = tc_i * 4 + i  # global j-tile index (b*NT + jt)
                    nc.tensor.matmul(
                        vtp[:, i * (DH + 1) : (i + 1) * (DH + 1)],
                        vT65[:, g * P : (g + 1) * P],
                        ident_sb[0 : DH + 1, 0 : DH + 1],
                        start=True,
                        stop=True,
                        skip_group_check=True,
                    )
                nc.vector.tensor_copy(
                    V65[:, tc_i * 4 * (DH + 1) : (tc_i + 1) * 4 * (DH + 1)],
                    vtp[:, :],
                )

            def q_chunk(hp):
                qps = pspool.tile([P, TOK_OWN], f32, tag="sT", name=f"q{hp}")
                for fc in range(8):
                    nc.tensor.matmul(
                        qps[:, :],
                        Wq_sb[:, fc * INNER + hp * P : fc * INNER + (hp + 1) * P],
                        xTo_sb[:, fc * TOK_OWN : (fc + 1) * TOK_OWN],
                        start=(fc == 0),
                        stop=(fc == 7),
                    )
                nc.vector.tensor_copy(
                    qT_sb[0:DH, (2 * hp) * TOK_OWN : (2 * hp + 1) * TOK_OWN],
                    qps[0:DH, :],
                )
                nc.vector.tensor_copy(
                    qT_sb[0:DH, (2 * hp + 1) * TOK_OWN : (2 * hp + 2) * TOK_OWN],
                    qps[DH:P, :],
                )

            kv_chunk(0)
            kv_chunk(1)
            q_chunk(0)
            q_chunk(1)
            kv_chunk(2)
            q_chunk(2)
            q_chunk(3)
            kv_chunk(3)

            # heads 4-7 duplicated into partitions 64:127 (for the
            # concurrent row-tile scores matmul)
            nc.vector.tensor_copy(
                qT_sb[DH:P, 0 : 4 * TOK_OWN], qT_sb[0:DH, 4 * TOK_OWN : 8 * TOK_OWN]
            )

            # batch-1 kv chunks injected between early batch-0 pairs
            late_kv = {8: 4, 12: 5, 16: 6, 20: 7}

            # ---- attention, software-pipelined one pair deep ----
            qT3 = qT_sb[:, :].rearrange("p (h t) -> p h t", h=HEADS)
            pairs = []
            for b in range(B):
                for sl, ext in ((0, EXTA), (1, EXTB)):
                    for jt in range(ext):
                        pairs.append((b, sl, ext, jt))
            pv_tiles = {}
            bias_tiles = {}

            def emit_scores(idx):
                b, sl, ext, jt = pairs[idx]
                if idx in late_kv:
                    kv_chunk(late_kv[idx])
                pair = jt if sl == 0 else EXTA + jt
                if pair % 4 == 0:
                    ci = b * NCHUNK + pair // 4
                    bias_sb = bpool.tile([P, 4 * HEADS * P], bf16, tag="bias")
                    eng = nc.scalar if ci % 2 == 0 else nc.sync
                    eng.dma_start(out=bias_sb[:], in_=bias_t[b, pair // 4])
                    bias_tiles[ci] = bias_sb
                bias_sb = bias_tiles[b * NCHUNK + pair // 4]
                boff = (pair % 4) * HEADS * P
                if (b, sl) not in pv_tiles:
                    pv_tiles[(b, sl)] = pspool.tile(
                        [P, HEADS * P], f32, tag="pv", name=f"pv{b}{sl}"
                    )
                qcol = (2 * b + sl) * P
                kcol = (b * NT + jt) * P
                sT = pspool.tile(
                    [P, HEADS * P], f32, tag="sT", name=f"sT{b}{sl}{jt}"
                )
                # scores: two CONCURRENT K=64 row-tiles
                nc.tensor.matmul(
                    sT[:, 0:512],
                    kT2[0:DH, kcol : kcol + P],
                    qT3[0:DH, 0:4, qcol : qcol + P],
                    start=True,
                    stop=True,
                    skip_group_check=True,
                )
                nc.tensor.matmul(
                    sT[:, 512:1024],
                    kT2[DH:P, kcol : kcol + P],
                    qT3[DH:P, 0:4, qcol : qcol + P],
                    start=True,
                    stop=True,
                    skip_group_check=True,
                )
                # softmax numerator: P^T = exp(sT/8) * expB in bf16
                pt_sb = ptpool.tile([P, HEADS * P], bf16, tag="pt")
                nc.scalar.activation(pt_sb[:, :], sT[:, :], AF.Exp, scale=0.125)
                ptm = ptpool.tile([P, HEADS * P], bf16, tag="pt", name=f"pm{idx}")
                nc.vector.tensor_mul(
                    ptm[:, :], pt_sb[:, :], bias_sb[:, boff : boff + HEADS * P]
                )
                return ptm

            def emit_pv(idx, ptm):
                b, sl, ext, jt = pairs[idx]
                pv = pv_tiles[(b, sl)]
                g = (b * NT + jt) * (DH + 1)
                for half in range(2):
                    nc.tensor.matmul(
                        pv[0 : DH + 1, half * 512 : (half + 1) * 512],
                        V65[:, g : g + DH + 1],
                        ptm[:, half * 512 : (half + 1) * 512],
                        start=(jt == 0),
                        stop=(jt == ext - 1),
                        skip_group_check=True,
                    )

            def emit_finish(b, sl):
                # normalize + output projection for a finished slot
                pv = pv_tiles.pop((b, sl))
                recip = cpool.tile(
                    [1, HEADS * P], f32, name=f"rc{b}{sl}", tag="recip", bufs=2
                )
                lg = cpool.tile(
                    [1, HEADS * P], f32, name=f"lg{b}{sl}", tag="lg", bufs=2
                )
                nc.scalar.activation(lg[:, :], pv[DH : DH + 1, :], AF.Ln)
                nc.scalar.activation(recip[:, :], lg[:, :], AF.Exp, scale=-1.0)
                bc_sb = ptpool.tile(
                    [DH, HEADS * P], f32, tag="bc", name=f"bc{b}{sl}", bufs=2
                )
                nc.gpsimd.partition_broadcast(bc_sb[:, :], recip[:, :])
                attnT = atpool.tile([P, HEADS * P], bf16, tag="at")
                for half in range(2):
                    fs = slice(half * 512, (half + 1) * 512)
                    nc.vector.tensor_mul(attnT[0:DH, fs], pv[0:DH, fs], bc_sb[:, fs])
                nc.vector.tensor_copy(
                    attnT[DH:P, 0 : 7 * P], attnT[0:DH, P : HEADS * P]
                )
                orow = (2 * b + sl) * P
                for half in range(2):
                    fs = slice(half * 512, (half + 1) * 512)
                    ops = pspool.tile(
                        [P, 512], f32, tag="pv", name=f"op{b}{sl}{half}"
                    )
                    nc.tensor.matmul(
                        ops[:, :], ones128[:, :], bo_sb[:, fs], start=True, stop=False
                    )
                    for hp in range(4):
                        nc.tensor.matmul(
                            ops[:, :],
                            attnT[:, 2 * hp * P : (2 * hp + 1) * P],
                            Wo_sb[:, hp * DIM + half * 512 : hp * DIM + (half + 1) * 512],
                            start=False,
                            stop=(hp == 3),
                        )
                    ob_sb = obpool.tile([P, 512], f32, tag="ob")
                    nc.vector.tensor_copy(ob_sb[:, :], ops[:, :])
                    nc.sync.dma_start(
                        out=out_t[orow : orow + P, half * 512 : (half + 1) * 512],
                        in_=ob_sb[:, :],
                    )

            pending = []  # [(due_idx, (b, sl))] finish work, deferred
            ptm_prev = None
            for idx in range(len(pairs)):
                ptm = emit_scores(idx)
                if idx > 0:
                    emit_pv(idx - 1, ptm_prev)
                    pb, psl, pext, pjt = pairs[idx - 1]
                    if pjt == pext - 1:
                        pending.append((idx + 1, (pb, psl)))
                ptm_prev = ptm
                while pending and pending[0][0] <= idx:
                    _, key = pending.pop(0)
                    emit_finish(*key)
            emit_pv(len(pairs) - 1, ptm_prev)
            for _, key in pending:
                emit_finish(*key)
            emit_finish(pairs[-1][0], pairs[-1][1])

    nc.compile()
    return nc


def prep_inputs(x, rel_pos_bias, Wq, Wkv, Wo, bo):
    """Build the 8 per-core input maps (host-side sharding/marshalling)."""
    x = np.asarray(x, dtype=np.float32)
    rel_pos_bias = np.asarray(rel_pos_bias, dtype=np.float32)
    bo = np.asarray(bo, dtype=np.float32).reshape(1, DIM)

    def chunk_pm(w, nf):
        # [nf*128, C] -> [128, nf*C] (partition-major chunks)
        w = np.asarray(w, dtype=np.float32)
        c = w.shape[1]
        return np.ascontiguousarray(
            w.reshape(nf, P, c).transpose(1, 0, 2).reshape(P, nf * c)
        ).astype(BF16)

    WqC = chunk_pm(Wq, 8)
    WkvC = chunk_pm(Wkv, 8)
    WoC = chunk_pm(Wo, 4)
    xT = np.concatenate([x[b].T for b in range(B)], axis=1)  # [1024, 4096]
    # [tc, p, fc*512+t] = xT[fc*128+p, tc*512+t]
    xTaC = np.ascontiguousarray(
        xT.reshape(8, P, 8, 512).transpose(2, 1, 0, 3).reshape(NTC, P, 8 * 512)
    ).astype(BF16)
    ident = np.eye(P, dtype=BF16)
    ones_r = np.ones((1, P), np.float32)
    ones_bn = np.ones((1, BN), dtype=BF16)

    ji = np.arange(N)  # global key index
    in_maps = []
    for c in range(NCORES):
        tiles = _q_tiles(c)
        # own tokens, order [b0A, b0B, b1A, b1B]
        xs = [x[b, t * P : (t + 1) * P, :] for b in range(B) for t in tiles]
        xToC = chunk_pm(np.concatenate(xs, axis=0).T, 8)

        biasT = np.zeros((B, NPAIR, P, HEADS, P), dtype=np.float32)
        for b in range(B):
            for sl, (t, ext) in enumerate(zip(tiles, (EXTA, EXTB))):
                qg = t * P + np.arange(P)  # global q index [128]
                nj = ext * P
                # [h, q, j] -> [jt, j, h, q], expB = exp(bias), 0 if masked
                blk = rel_pos_bias[:, t * P : (t + 1) * P, :nj]
                blk = np.exp(blk.reshape(HEADS, P, ext, P).transpose(2, 3, 0, 1))
                m = ji[:nj, None] > qg[None, :]  # [j, q] masked
                blk = np.where(
                    m.reshape(ext, P, 1, P).repeat(HEADS, axis=2)[:, :, :HEADS, :],
                    0.0,
                    blk,
                )
                base = 0 if sl == 0 else EXTA
                biasT[b, base : base + ext] = blk
        # [b, pair, j, h, q] -> [b, chunk, j, (pair%4, h, q)]
        biasC = np.ascontiguousarray(
            biasT.reshape(B, NCHUNK, 4, P, HEADS, P)
            .transpose(0, 1, 3, 2, 4, 5)
            .reshape(B, NCHUNK, P, 4 * HEADS * P)
        ).astype(BF16)
        in_maps.append(
            {
                "xTaC": xTaC,
                "xToC": xToC,
                "WqC": WqC,
                "WkvC": WkvC,
                "WoC": WoC,
                "bo": bo,
                "ident": ident,
                "ones_r": ones_r,
                "ones_bn": ones_bn,
                "biasC": biasC,
            }
        )
    return in_maps


def assemble(outs):
    """outs: list of 8 [512, 1024] arrays -> full [2, 2048, 1024]."""
    full = np.empty((B, N, DIM), dtype=np.float32)
    for c in range(NCORES):
        o = np.asarray(outs[c])
        for b in range(B):
            for sl, t in enumerate(_q_tiles(c)):
                full[b, t * P : (t + 1) * P, :] = o[
                    (2 * b + sl) * P : (2 * b + sl + 1) * P
                ]
    return full


def kernel(**inputs):
    from concourse.bass_utils import run_bass_kernel_spmd

    if "nc" not in _CACHE:
        _CACHE["nc"] = build_graph()
    nc = _CACHE["nc"]
    in_maps = prep_inputs(
        inputs["x"], inputs["rel_pos_bias"], inputs["Wq"], inputs["Wkv"],
        inputs["Wo"], inputs["bo"],
    )
    res = run_bass_kernel_spmd(
        nc, in_maps, core_ids=list(range(NCORES)),
        trace=bool(int(os.environ.get("KERNEL_TRACE", "0"))),
    )
    _CACHE["last_results"] = res
    return assemble([r["out"] for r in res.results])
